# revision 28
# baseline (speedup 1.0000x reference)
"""Trainium2 Bass kernel for DoubleAttention (nn_DoubleAttention_82703890252117).

Reference computation (per batch element b, n = H*W = 4096, c = 512, d = v = 256):
    q = x @ Wq + bq                      # [n, d]
    k = x @ Wk + bk                      # [n, v]
    v_ = x @ Wv + bv                     # [n, v]
    am = softmax(k, axis=n)              # per-channel softmax over positions
    av = softmax(v_, axis=n)
    gd = am^T @ q                        # [v, d]
    out = av @ gd                        # [n, d]
    y = out @ Wr + br                    # [n, c]

Algebraic restructuring used here (exact in real arithmetic):
  * softmax over n is invariant to the per-channel constants bk, bv -> drop them.
  * am = e_k / s_k with e_k = exp(x@Wk), s_k[v] = sum_n e_k[n,v] (no max-sub
    needed: k has std ~0.45, exp is tame).
  * sum_n am[n,v] = 1  =>  bq folds into gd:  gd = (e_k^T @ (x@Wq))/s_k + bq.
  * out @ Wr = e_v @ P with P = G @ Wr, G[v,:] = gd[v,:]/s_v[v].
  So:  y = e_v @ P + br, and the only softmax normalizations are row scalings
  of the tiny [256,256] descriptor matrix.

Per-core work (data parallel over batch, 2 batch elements per core):
  phase A (per 512-row chunk of x): DMA x, PE-transpose to x^T (f32r, exact
    permutation), fused QK matmul -> q (DVE copy) and e_k (ACT exp) evictions,
    gd accumulation in PSUM (a ones-column in q produces s_k for free),
    e_v^T production with ACT exp + accum_out partial sums for s_v.
  phase B: tiny [256,256] normalization + transpose + P = G @ Wr.
  phase C: y = e_v @ P (+ br), DMA'd straight from PSUM per 128-row tile.
  Emission order A0 B0 A1 C0 B1 C1 hides the phase-B bubble of batch 0 under
  batch 1's phase A.

All matmuls run as float32r (full PE speed at moving-dim >= 256, ~1e-4 rel
precision); PSUM accumulation is fp32.
"""

import sys

if "/opt/trn_rl_repo" not in sys.path:
    sys.path.insert(0, "/opt/trn_rl_repo")

import numpy as np

B, H, W, C = 16, 64, 64, 512
DIM, VDIM = 256, 256
N_CORES = 8
B_LOC = B // N_CORES          # batch elements per core
N = H * W                     # 4096 positions per batch element
ROWS = B_LOC * N              # 8192 rows of x per core
CHUNK = 512                   # n-rows per phase-A chunk
N_CHUNKS = N // CHUNK         # 8
N_SUB = CHUNK // 128          # 4 sub-tiles per chunk
N_CT = C // 128               # 4 contraction tiles over c
N_VT = VDIM // 128            # 2
N_DT = DIM // 128             # 2
N_NT = N // 128               # 32 row-tiles per batch
QK = DIM + VDIM               # 512 fused q|k output width


def _ts(i, sz):
    return slice(i * sz, (i + 1) * sz)


def _build(with_bq, with_br):
    import concourse.bass as bass
    import concourse.mybir as mybir
    from concourse import bacc
    from concourse.tile import TileContext

    f32 = mybir.dt.float32
    f32r = mybir.dt.float32r
    AF = mybir.ActivationFunctionType
    AX = mybir.AxisListType

    nc = bacc.Bacc("TRN2", target_bir_lowering=False, debug=False,
                   num_devices=N_CORES)

    x = nc.declare_dram_parameter("x", [ROWS, C], f32, isOutput=False)
    wq = nc.declare_dram_parameter("Wq", [C, DIM], f32, isOutput=False)
    wk = nc.declare_dram_parameter("Wk", [C, VDIM], f32, isOutput=False)
    wv = nc.declare_dram_parameter("Wv", [C, VDIM], f32, isOutput=False)
    wr = nc.declare_dram_parameter("Wr", [DIM, C], f32, isOutput=False)
    ones = nc.declare_dram_parameter("ones", [128, N_NT, 2], f32, isOutput=False)
    idin = nc.declare_dram_parameter("ident", [128, 128], f32, isOutput=False)
    if with_bq:
        bq = nc.declare_dram_parameter("bq", [DIM], f32, isOutput=False)
    if with_br:
        br = nc.declare_dram_parameter("br", [C], f32, isOutput=False)
    out = nc.declare_dram_parameter("out", [ROWS, C], f32, isOutput=True)

    # the bias variants carry extra broadcast/temp tiles; give the room back
    # from pools that only matter for peak overlap
    slim = with_bq or with_br
    with TileContext(nc) as tc:
        with tc.tile_pool(name="const", bufs=1) as cpool, \
             tc.tile_pool(name="xin", bufs=2) as xin, \
             tc.tile_pool(name="xt", bufs=8) as xtp, \
             tc.tile_pool(name="ek", bufs=5 if slim else 6) as ekp, \
             tc.tile_pool(name="qa", bufs=1 if slim else 2) as qap, \
             tc.tile_pool(name="ev", bufs=2) as evp, \
             tc.tile_pool(name="sm", bufs=2) as sm, \
             tc.tile_pool(name="osb", bufs=4 if slim else 6) as osbp, \
             tc.tile_pool(name="pio", bufs=6, space="PSUM") as pio, \
             tc.tile_pool(name="pgd", bufs=2, space="PSUM") as pgd:

            # ---- constants ----
            ident = cpool.tile([128, 128], f32r, tag="ident")
            nc.scalar.dma_start(ident[:], idin[:].bitcast(f32r))
            ident32 = cpool.tile([128, 128], f32, tag="ident32")
            nc.scalar.dma_start(ident32[:], idin[:])
            wqk_t = []
            wv_t = []

            def load_qkv_weights():
                for ct in range(N_CT):
                    t = cpool.tile([128, QK], f32r, tag=f"wqk{ct}",
                                   name=f"wqk_t{ct}")
                    nc.sync.dma_start(t[:, 0:DIM],
                                      wq[_ts(ct, 128), :].bitcast(f32r))
                    nc.sync.dma_start(t[:, DIM:QK],
                                      wk[_ts(ct, 128), :].bitcast(f32r))
                    wqk_t.append(t)
                for ct in range(N_CT):
                    t = cpool.tile([128, VDIM], f32r, tag=f"wv{ct}",
                                   name=f"wv_t{ct}")
                    nc.sync.dma_start(t[:], wv[_ts(ct, 128), :].bitcast(f32r))
                    wv_t.append(t)
            wr_t = []

            def load_wr():
                for dt_ in range(N_DT):
                    t = cpool.tile([128, C], f32r, tag=f"wr{dt_}",
                                   name=f"wr_t{dt_}")
                    nc.scalar.dma_start(t[:], wr[_ts(dt_, 128), :].bitcast(f32r))
                    wr_t.append(t)
            if with_bq:
                bq_b = cpool.tile([128, DIM], f32, tag="bqb")
                nc.sync.dma_start(bq_b[:], bq[None, :].broadcast_to([128, DIM]))
            if with_br:
                br_b = cpool.tile([128, C], f32, tag="brb")
                nc.sync.dma_start(br_b[:], br[None, :].broadcast_to([128, C]))

            state = {}

            def phase_a(b, chunks=None):
                base = b * N
                if chunks is None:
                    chunks = range(N_CHUNKS)
                q_all = qap.tile([128, N_NT, DIM + 2], f32r, tag="q_all",
                                 name=f"q_all{b}")
                ev_t = [evp.tile([128, N], f32r, tag=f"evT{vt}",
                                 name=f"evT{b}_{vt}")
                        for vt in range(N_VT)]
                gd_ps = [pgd.tile([128, DIM + 2], f32, tag="gd",
                                  name=f"gd{b}_{vt}")
                         for vt in range(N_VT)]
                svp = [sm.tile([128, N_CHUNKS], f32, tag=f"svp{vt}",
                               name=f"svp{b}_{vt}")
                       for vt in range(N_VT)]
                state[b] = (q_all, ev_t, gd_ps, svp)
                phase_a_chunks(b, list(chunks))

            def phase_a_chunks(b, chunks):
                base = b * N
                q_all, ev_t, gd_ps, svp = state[b]
                for ch in chunks:
                    r0 = base + ch * CHUNK
                    xch = xin.tile([128, N_SUB, C], f32r, tag="xch",
                                   name=f"xch{b}_{ch}")
                    nc.sync.dma_start(
                        xch[:],
                        x[r0:r0 + CHUNK, :].bitcast(f32r)
                        .rearrange("(s p) c -> p s c", p=128),
                    )
                    if ch == chunks[0]:
                        if b == 0 and not wqk_t:
                            load_qkv_weights()
                        nc.sync.dma_start(q_all[:, :, DIM:DIM + 2],
                                          ones[:].bitcast(f32r))
                    # transpose x chunk -> xT tiles [c128, n512] (f32r, 1.5cyc/row)
                    xt_t = []
                    for ct in range(N_CT):
                        pxt = pio.tile([128, CHUNK], f32r, tag="io",
                                       name=f"pxt{b}_{ch}_{ct}")
                        for s in range(N_SUB):
                            nc.tensor.transpose(pxt[:, _ts(s, 128)],
                                                xch[:, s, _ts(ct, 128)],
                                                ident[:])
                        t = xtp.tile([128, CHUNK], f32r, tag="xt",
                                     name=f"xt{b}_{ch}_{ct}")
                        nc.vector.tensor_copy(t[:], pxt[:].bitcast(f32))
                        xt_t.append(t)
                    # fused q|k; gd matmuls staggered one subtile behind so the
                    # ACT ek/q evictions hide under the next subtile's qk work
                    def emit_gd(nt):
                        for vt in range(N_VT):
                            nc.tensor.matmul(
                                gd_ps[vt][:], gd_ek[nt][:, _ts(vt, 128)],
                                q_all[:, nt, :],
                                start=(nt == 0),
                                stop=(nt == N_NT - 1))

                    def emit_ev():
                        # e_v^T production (+ s_v partials via accum_out)
                        for vt in range(N_VT):
                            pev = pio.tile([128, CHUNK], f32, tag="io",
                                           name=f"pev{b}_{ch}_{vt}")
                            for ct in range(N_CT):
                                nc.tensor.matmul(pev[:],
                                                 wv_t[ct][:, _ts(vt, 128)],
                                                 xt_t[ct][:],
                                                 start=(ct == 0),
                                                 stop=(ct == N_CT - 1))
                            nc.scalar.activation(ev_t[vt][:, _ts(ch, CHUNK)],
                                                 pev[:], AF.Exp,
                                                 accum_out=svp[vt][:, ch:ch + 1])

                    last_chunk = (ch == N_CHUNKS - 1)
                    if last_chunk:
                        # ev first so its ACT eviction (-> s_v partial) is not
                        # queued behind the chunk's ek/q evictions: phase-B
                        # stats are gated on it.
                        emit_ev()
                    gd_ek = {}
                    for s in range(N_SUB):
                        nt = ch * N_SUB + s
                        pqk = pio.tile([128, QK], f32, tag="io",
                                       name=f"pqk{b}_{nt}")
                        for ct in range(N_CT):
                            nc.tensor.matmul(pqk[:], xt_t[ct][:, _ts(s, 128)],
                                             wqk_t[ct][:],
                                             start=(ct == 0),
                                             stop=(ct == N_CT - 1))
                        nc.scalar.activation(q_all[:, nt, 0:DIM],
                                             pqk[:, 0:DIM], AF.Copy)
                        ek = ekp.tile([128, VDIM], f32r, tag="ek",
                                      name=f"ek{b}_{nt}")
                        nc.scalar.activation(ek[:], pqk[:, DIM:QK], AF.Exp)
                        gd_ek[nt] = ek
                        if s > 0:
                            emit_gd(nt - 1)
                    if not last_chunk:
                        emit_ev()
                    # last subtile's gd after the ev matmuls (same hiding)
                    emit_gd(ch * N_SUB + N_SUB - 1)

            def phase_b_stats(b):
                _, ev_t, gd_ps, svp = state[b]
                g_t = []
                for vt in range(N_VT):
                    sv = sm.tile([128, 1], f32, tag=f"sv{vt}",
                                 name=f"sv{b}_{vt}")
                    nc.vector.reduce_sum(sv[:], svp[vt][:], axis=AX.X)
                    prod = sm.tile([128, 1], f32, tag=f"prod{vt}",
                                   name=f"prod{b}_{vt}")
                    nc.vector.tensor_mul(prod[:], gd_ps[vt][:, DIM:DIM + 1],
                                         sv[:])
                    r_ = sm.tile([128, 1], f32, tag=f"r{vt}",
                                 name=f"r{b}_{vt}")
                    nc.vector.reciprocal(r_[:], prod[:])
                    g = sm.tile([128, DIM], f32, tag=f"g{vt}",
                                name=f"g{b}_{vt}", bufs=1)
                    nc.vector.tensor_scalar_mul(g[:], gd_ps[vt][:, 0:DIM], r_[:])
                    if with_bq:
                        rv = sm.tile([128, 1], f32, tag=f"rv{vt}",
                                     name=f"rv{b}_{vt}")
                        nc.vector.reciprocal(rv[:], sv[:])
                        tmp = sm.tile([128, DIM], f32, tag=f"tmp{vt}",
                                      name=f"tmp{b}_{vt}")
                        nc.vector.tensor_scalar_mul(tmp[:], bq_b[:], rv[:])
                        nc.vector.tensor_add(g[:], g[:], tmp[:])
                    g_t.append(g)
                state[b] = (ev_t, g_t)

            def phase_b_mm(b):
                ev_t, g_t = state[b]
                gt_t, p_t = [], []
                for dt_ in range(N_DT):
                    pgt = pio.tile([128, VDIM], f32, tag="io",
                                   name=f"pgt{b}_{dt_}")
                    for vt in range(N_VT):
                        nc.tensor.transpose(pgt[:, _ts(vt, 128)],
                                            g_t[vt][:, _ts(dt_, 128)],
                                            ident32[:])
                    gt = sm.tile([128, VDIM], f32r, tag=f"gt{dt_}",
                                 name=f"gt{b}_{dt_}", bufs=1)
                    nc.scalar.activation(gt[:], pgt[:], AF.Copy)
                    gt_t.append(gt)
                for vt in range(N_VT):
                    pp = pio.tile([128, C], f32, tag="io",
                                  name=f"pp{b}_{vt}")
                    for dt_ in range(N_DT):
                        nc.tensor.matmul(pp[:], gt_t[dt_][:, _ts(vt, 128)],
                                         wr_t[dt_][:],
                                         start=(dt_ == 0),
                                         stop=(dt_ == N_DT - 1))
                    p = sm.tile([128, C], f32r, tag=f"p{vt}",
                                name=f"p{b}_{vt}", bufs=1)
                    nc.scalar.activation(p[:], pp[:], AF.Copy)
                    p_t.append(p)
                state[b] = (ev_t, p_t)

            def phase_c(b, tiles=None):
                base = b * N
                ev_t, p_t = state[b]
                if tiles is None:
                    tiles = range(N_NT)
                for nt in tiles:
                    pf = pio.tile([128, C], f32, tag="io",
                                  name=f"pf{b}_{nt}")
                    for vt in range(N_VT):
                        nc.tensor.matmul(pf[:], ev_t[vt][:, _ts(nt, 128)],
                                         p_t[vt][:],
                                         start=(vt == 0),
                                         stop=(vt == N_VT - 1))
                    o = osbp.tile([128, C], f32, tag="o", name=f"o{b}_{nt}")
                    if with_br:
                        nc.vector.tensor_add(o[:], pf[:], br_b[:])
                    elif nt % 2 == 0:
                        nc.vector.tensor_copy(o[:], pf[:])
                    else:
                        nc.scalar.activation(o[:], pf[:], AF.Copy)
                    nc.sync.dma_start(
                        out[base + nt * 128:base + (nt + 1) * 128, :], o[:])

            phase_a(0)
            load_wr()
            phase_b_stats(0)        # DVE chain drains while A1c0's PE work runs
            phase_a(1, chunks=[0])
            phase_b_mm(0)           # PE bits: g already evicted by now
            phase_a_chunks(1, [1])
            RESERVE = 8
            c0_tiles = list(range(N_NT - RESERVE))
            for i, ch in enumerate(range(2, N_CHUNKS)):
                lo = (i * len(c0_tiles)) // (N_CHUNKS - 2)
                hi = ((i + 1) * len(c0_tiles)) // (N_CHUNKS - 2)
                phase_a_chunks(1, [ch])
                phase_c(0, tiles=c0_tiles[lo:hi])
            phase_b_stats(1)
            phase_c(0, tiles=list(range(N_NT - RESERVE, N_NT)))
            phase_b_mm(1)           # PE bits hidden behind the C0 reserve tiles
            phase_c(1)
    nc.compile()
    return nc


def _build_fast():
    """Zero-bias fast path.

    Restructure vs the generic build: q is never materialized. Using
      gd = (ek^T x) Wq       (exact in real arithmetic)
    the q-projection (32.8K PE cyc) + gd einsum (16.5K) become
      gdxT[c,v] = sum_n x[n,c] ek[n,v]   (x-natural stationary, ek moving;
                                          32.8K cyc, accumulated in PSUM
                                          across all 32 row-tiles)
      gd^T = Wq^T @ gdxT                 (2K cyc, no transposes needed:
                                          Wq natural stationary, gdxT moving)
    s_k rides on 1-wide matmuls against a ones column (ek stationary reused).
    P = gd@Wr picks up the 1/(s_k*s_v) row scaling for free via the ACT
    eviction's per-partition scale operand.
    """
    import concourse.bass as bass
    import concourse.mybir as mybir
    from concourse import bacc
    from concourse.tile import TileContext

    f32 = mybir.dt.float32
    f32r = mybir.dt.float32r
    AF = mybir.ActivationFunctionType
    AX = mybir.AxisListType

    nc = bacc.Bacc("TRN2", target_bir_lowering=False, debug=False,
                   num_devices=N_CORES)

    x = nc.declare_dram_parameter("x", [ROWS, C], f32, isOutput=False)
    wq = nc.declare_dram_parameter("Wq", [C, DIM], f32, isOutput=False)
    wk = nc.declare_dram_parameter("Wk", [C, VDIM], f32, isOutput=False)
    wv = nc.declare_dram_parameter("Wv", [C, VDIM], f32, isOutput=False)
    wr = nc.declare_dram_parameter("Wr", [DIM, C], f32, isOutput=False)
    idin = nc.declare_dram_parameter("ident", [128, 128], f32, isOutput=False)
    out = nc.declare_dram_parameter("out", [ROWS, C], f32, isOutput=True)

    with TileContext(nc) as tc:
        with tc.tile_pool(name="const", bufs=1) as cpool, \
             tc.tile_pool(name="xin", bufs=3) as xin, \
             tc.tile_pool(name="xb", bufs=3) as xbp, \
             tc.tile_pool(name="xt", bufs=8) as xtp, \
             tc.tile_pool(name="ek", bufs=5) as ekp, \
             tc.tile_pool(name="ev", bufs=2) as evp, \
             tc.tile_pool(name="sm", bufs=2) as sm, \
             tc.tile_pool(name="osb", bufs=4) as osbp, \
             tc.tile_pool(name="pio", bufs=5, space="PSUM") as pio, \
             tc.tile_pool(name="pgd", bufs=2, space="PSUM") as pgd, \
             tc.tile_pool(name="psk", bufs=1, space="PSUM") as psk:

            bf16 = mybir.dt.bfloat16

            ident = cpool.tile([128, 128], f32r, tag="ident")
            nc.scalar.dma_start(ident[:], idin[:].bitcast(f32r))
            identb = cpool.tile([128, 128], bf16, tag="identb")
            nc.vector.tensor_copy(identb[:], ident[:].bitcast(f32))
            # 2-wide (not 1): fp32r matmuls need even moving/dst inner counts
            ones1 = cpool.tile([128, 2], f32, tag="ones1")
            nc.vector.memset(ones1[:], 1.0)

            wk_t, wv_t, wq_t, wr_t = [], [], [], []

            wkf_t, wvf_t = [], []

            def load_wkv():
                for ts_, w in ((wkf_t, wk), (wvf_t, wv)):
                    i = 0 if w is wk else 1
                    for ct in range(N_CT):
                        t = cpool.tile([128, VDIM], f32r, tag=f"wf{i}_{ct}",
                                       name=f"wf{i}_{ct}")
                        nc.scalar.dma_start(t[:], w[_ts(ct, 128), :].bitcast(f32r))
                        ts_.append(t)
                for i, (ts_, st_) in enumerate(((wk_t, wkf_t), (wv_t, wvf_t))):
                    for ct in range(N_CT):
                        t = cpool.tile([128, VDIM], bf16, tag=f"wb{i}_{ct}",
                                       name=f"wb{i}_{ct}")
                        nc.vector.tensor_copy(t[:], st_[ct][:].bitcast(f32))
                        ts_.append(t)

            def load_wq(cts):
                for ct in cts:
                    t = cpool.tile([128, DIM], f32r, tag=f"wq{ct}",
                                   name=f"wq_t{ct}")
                    nc.sync.dma_start(t[:], wq[_ts(ct, 128), :].bitcast(f32r))
                    wq_t.append(t)

            def load_wr():
                for dt_ in range(N_DT):
                    t = cpool.tile([128, C], f32r, tag=f"wr{dt_}",
                                   name=f"wr_t{dt_}")
                    nc.sync.dma_start(t[:], wr[_ts(dt_, 128), :].bitcast(f32r))
                    wr_t.append(t)

            state = {}

            def phase_a_start(b):
                state[b] = dict(
                    ev=[evp.tile([128, N], f32r, tag=f"evT{vt}",
                                 name=f"evT{b}_{vt}") for vt in range(N_VT)],
                    svp=[sm.tile([128, N_CHUNKS], f32, tag=f"svp{vt}",
                                 name=f"svp{b}_{vt}") for vt in range(N_VT)],
                    gdx=[pgd.tile([128, 2, VDIM], f32, tag="gdxT",
                                  name=f"gdxT{b}_{i}") for i in range(2)],
                    sk=psk.tile([128, 4], f32, tag="sk", name=f"sk{b}"),
                    ek={}, xch={})

            def emit_gdx(b, nt):
                st = state[b]
                ek = st["ek"].pop(nt)
                xch = st["xch"][nt // N_SUB]
                s = nt % N_SUB
                first, last = (nt == 0), (nt == N_NT - 1)
                # each PSUM bank is one 2KB zero region = ONE accumulation
                # group: start only on the bank's first write (a start marks
                # the whole region pending-zero, so a second start= chain in
                # the same bank would clobber the first), stop on its last.
                for cb in range(N_CT):
                    nc.tensor.matmul(st["gdx"][cb // 2][:, cb % 2, :],
                                     xch[:, s, _ts(cb, 128)], ek[:],
                                     start=first and cb % 2 == 0,
                                     stop=last and cb % 2 == 1,
                                     skip_group_check=True)
                for vt in range(N_VT):
                    nc.tensor.matmul(st["sk"][:, 2 * vt:2 * vt + 2],
                                     ek[:, _ts(vt, 128)], ones1[:].bitcast(f32r),
                                     start=first and vt == 0,
                                     stop=last and vt == 1,
                                     skip_group_check=True)

            def phase_a_chunk(b, ch):
                st = state[b]
                r0 = b * N + ch * CHUNK
                xch = xin.tile([128, N_SUB, C], f32r, tag="xch",
                               name=f"xch{b}_{ch}")
                if b == 0 and ch == 0:
                    # warmup: two half-c DMAs so the first transposes start
                    # after ~half the chunk has landed
                    for h in range(2):
                        nc.sync.dma_start(
                            xch[:, :, _ts(h, 256)],
                            x[r0:r0 + CHUNK, _ts(h, 256)].bitcast(f32r)
                            .rearrange("(s p) c -> p s c", p=128))
                else:
                    nc.sync.dma_start(
                        xch[:],
                        x[r0:r0 + CHUNK, :].bitcast(f32r)
                        .rearrange("(s p) c -> p s c", p=128))
                st["xch"][ch] = xch
                if b == 0 and ch == 0:
                    load_wkv()
                warm = (b == 0 and ch == 0)
                if warm:
                    # first chunk: f32r transposes straight off the DMA'd
                    # data — keeps the Pool convert stage off the warmup
                    # critical path
                    src, idn, tdt = xch, ident, f32r
                    wks_t, wvs_t = wkf_t, wvf_t
                else:
                    # bf16 shadow of the chunk on the otherwise-idle Pool
                    # engine: bf16 transposes run 1.0 cyc/row vs 1.5 f32r
                    xchb = xbp.tile([128, N_SUB, C], bf16, tag="xchb",
                                    name=f"xchb{b}_{ch}")
                    nc.gpsimd.tensor_copy(xchb[:], xch[:].bitcast(f32))
                    src, idn, tdt = xchb, identb, bf16
                    wks_t, wvs_t = wk_t, wv_t

                def _w(ap):
                    return ap
                last_chunk = (ch == N_CHUNKS - 1)
                xt_t = []
                for ct in range(N_CT):
                    pxt = pio.tile([128, CHUNK], tdt, tag="io",
                                   name=f"pxt{b}_{ch}_{ct}")
                    for s in range(N_SUB):
                        nc.tensor.transpose(pxt[:, _ts(s, 128)],
                                            src[:, s, _ts(ct, 128)], idn[:])
                    t = xtp.tile([128, CHUNK], tdt, tag="xt",
                                 name=f"xt{b}_{ch}_{ct}")
                    if warm:
                        nc.vector.tensor_copy(t[:], pxt[:].bitcast(f32))
                    else:
                        nc.vector.tensor_copy(t[:], pxt[:])
                    xt_t.append(t)

                def emit_ev():
                    for vt in range(N_VT):
                        pev = pio.tile([128, CHUNK], f32, tag="io",
                                       name=f"pev{b}_{ch}_{vt}")
                        for ct in range(N_CT):
                            nc.tensor.matmul(pev[:],
                                             _w(wvs_t[ct][:, _ts(vt, 128)]),
                                             xt_t[ct][:],
                                             start=(ct == 0),
                                             stop=(ct == N_CT - 1))
                        nc.scalar.activation(
                            st["ev"][vt][:, _ts(ch, CHUNK)], pev[:], AF.Exp,
                            accum_out=st["svp"][vt][:, ch:ch + 1])

                # last chunk: ev first so its ACT eviction (-> s_v partial)
                # isn't queued behind the chunk's ek evictions; phase-B stats
                # gate on it. other chunks: ev after sub 1 so the pev PSUM
                # slots free before the next chunk's transposes need the ring.
                if last_chunk:
                    emit_ev()
                for s in range(N_SUB):
                    nt = ch * N_SUB + s
                    pk = pio.tile([128, VDIM], f32, tag="io",
                                  name=f"pk{b}_{nt}")
                    for ct in range(N_CT):
                        nc.tensor.matmul(pk[:], xt_t[ct][:, _ts(s, 128)],
                                         _w(wks_t[ct][:]),
                                         start=(ct == 0),
                                         stop=(ct == N_CT - 1))
                    ek = ekp.tile([128, VDIM], f32r, tag="ek",
                                  name=f"ek{b}_{nt}")
                    nc.scalar.activation(ek[:], pk[:], AF.Exp)
                    st["ek"][nt] = ek
                    if nt > 0:
                        emit_gdx(b, nt - 1)
                    if s == 1 and not last_chunk:
                        emit_ev()
                if last_chunk:
                    emit_gdx(b, N_NT - 1)

            def b_stats(b):
                st = state[b]
                r_t, gx_t = [], []
                for vt in range(N_VT):
                    sv = sm.tile([128, 1], f32, tag=f"sv{vt}",
                                 name=f"sv{b}_{vt}")
                    nc.vector.reduce_sum(sv[:], st["svp"][vt][:], axis=AX.X)
                    prod = sm.tile([128, 1], f32, tag=f"prod{vt}",
                                   name=f"prod{b}_{vt}")
                    nc.vector.tensor_mul(prod[:], st["sk"][:, 2 * vt:2 * vt + 1],
                                         sv[:])
                    r_ = sm.tile([128, 1], f32, tag=f"r{vt}",
                                 name=f"r{b}_{vt}")
                    nc.vector.reciprocal(r_[:], prod[:])
                    r_t.append(r_)
                for i in range(2):
                    gx = sm.tile([128, 2, VDIM], f32r, tag=f"gx{i}",
                                 name=f"gx{b}_{i}")
                    nc.scalar.activation(gx[:], st["gdx"][i][:], AF.Copy)
                    gx_t.append(gx)
                st["r"] = r_t
                st["gx"] = gx_t

            def b_mm1(b):
                st = state[b]
                gdT = []
                for dt_ in range(N_DT):
                    pg = pio.tile([128, VDIM], f32, tag="io",
                                  name=f"pg{b}_{dt_}")
                    for cb in range(N_CT):
                        nc.tensor.matmul(pg[:], wq_t[cb][:, _ts(dt_, 128)],
                                         st["gx"][cb // 2][:, cb % 2, :],
                                         start=(cb == 0),
                                         stop=(cb == N_CT - 1))
                    g = sm.tile([128, VDIM], f32r, tag=f"gdT{dt_}",
                                name=f"gdT{b}_{dt_}")
                    nc.scalar.activation(g[:], pg[:], AF.Copy)
                    gdT.append(g)
                st["gdT"] = gdT

            def b_mm2(b):
                st = state[b]
                p_t = []
                for vt in range(N_VT):
                    pp = pio.tile([128, C], f32, tag="io",
                                  name=f"pp{b}_{vt}")
                    for dt_ in range(N_DT):
                        nc.tensor.matmul(pp[:], st["gdT"][dt_][:, _ts(vt, 128)],
                                         wr_t[dt_][:],
                                         start=(dt_ == 0),
                                         stop=(dt_ == N_DT - 1))
                    p = sm.tile([128, C], f32r, tag=f"p{vt}",
                                name=f"p{b}_{vt}")
                    nc.scalar.activation(p[:], pp[:], AF.Copy,
                                         scale=st["r"][vt][:])
                    p_t.append(p)
                st["p"] = p_t

            def phase_c(b, pairs, singles_first=False):
                st = state[b]
                if singles_first and pairs:
                    # split the leading pair into single-tile DMAs so the
                    # tail's output-DMA pipe starts draining ~a tile earlier
                    for nt in pairs[0]:
                        o = osbp.tile([128, 1, C], f32, tag="o1",
                                      name=f"o1{b}_{nt}")
                        pf = pio.tile([128, C], f32, tag="io",
                                      name=f"pf{b}_{nt}")
                        for vt in range(N_VT):
                            nc.tensor.matmul(pf[:],
                                             st["ev"][vt][:, _ts(nt, 128)],
                                             st["p"][vt][:],
                                             start=(vt == 0),
                                             stop=(vt == N_VT - 1))
                        if nt % 2 == 0:
                            nc.vector.tensor_copy(o[:, 0, :], pf[:])
                        else:
                            nc.scalar.activation(o[:, 0, :], pf[:], AF.Copy)
                        r0 = b * N + nt * 128
                        nc.sync.dma_start(out[r0:r0 + 128, :], o[:, 0, :])
                    pairs = pairs[1:]
                for pr in pairs:
                    o = osbp.tile([128, 2, C], f32, tag="o",
                                  name=f"o{b}_{pr[0]}")
                    for j, nt in enumerate(pr):
                        pf = pio.tile([128, C], f32, tag="io",
                                      name=f"pf{b}_{nt}")
                        for vt in range(N_VT):
                            nc.tensor.matmul(pf[:],
                                             st["ev"][vt][:, _ts(nt, 128)],
                                             st["p"][vt][:],
                                             start=(vt == 0),
                                             stop=(vt == N_VT - 1))
                        if nt % 2 == 0:
                            nc.vector.tensor_copy(o[:, j, :], pf[:])
                        else:
                            nc.scalar.activation(o[:, j, :], pf[:], AF.Copy)
                    r0 = b * N + pr[0] * 128
                    nc.sync.dma_start(
                        out[r0:r0 + 256, :].rearrange("(s p) c -> p s c",
                                                      p=128),
                        o[:])

            phase_a_start(0)
            for ch in range(N_CHUNKS):
                phase_a_chunk(0, ch)
                if ch == 1:
                    load_wq([0, 1])
                elif ch == 2:
                    load_wq([2, 3])
                elif ch == 3:
                    load_wr()
            b_stats(0)
            phase_a_start(1)
            phase_a_chunk(1, 0)
            b_mm1(0)
            b_mm2(0)
            phase_a_chunk(1, 1)
            pairs = [(2 * i, 2 * i + 1) for i in range(N_NT // 2)]
            RES = 4
            main0 = pairs[:len(pairs) - RES]
            for i, ch in enumerate(range(2, N_CHUNKS)):
                phase_a_chunk(1, ch)
                lo = (i * len(main0)) // (N_CHUNKS - 2)
                hi = ((i + 1) * len(main0)) // (N_CHUNKS - 2)
                phase_c(0, main0[lo:hi])
            b_stats(1)
            phase_c(0, pairs[-RES:-2])
            b_mm1(1)
            phase_c(0, pairs[-2:-1])
            b_mm2(1)
            phase_c(0, pairs[-1:])
            phase_c(1, pairs)
    nc.compile()
    return nc


_NC_CACHE = {}


def kernel(**inputs):
    from concourse.bass_utils import run_bass_kernel_spmd

    x = np.ascontiguousarray(np.asarray(inputs["x"], dtype=np.float32))
    Wq = np.ascontiguousarray(np.asarray(inputs["Wq"], dtype=np.float32))
    Wk = np.ascontiguousarray(np.asarray(inputs["Wk"], dtype=np.float32))
    Wv = np.ascontiguousarray(np.asarray(inputs["Wv"], dtype=np.float32))
    Wr = np.ascontiguousarray(np.asarray(inputs["Wr"], dtype=np.float32))
    bq = np.asarray(inputs["bq"], dtype=np.float32)
    br = np.asarray(inputs["br"], dtype=np.float32)
    # bk/bv shift per-channel constants into the position-softmax -> no effect.

    with_bq = bool(np.any(bq))
    with_br = bool(np.any(br))
    fast = not (with_bq or with_br)
    key = "fast" if fast else (with_bq, with_br)
    if key not in _NC_CACHE:
        _NC_CACHE[key] = _build_fast() if fast else _build(with_bq, with_br)
    nc = _NC_CACHE[key]

    xs = x.reshape(B, N, C)
    in_maps = []
    for i in range(N_CORES):
        m = {
            "x": np.ascontiguousarray(
                xs[i * B_LOC:(i + 1) * B_LOC].reshape(ROWS, C)),
            "Wq": Wq, "Wk": Wk, "Wv": Wv, "Wr": Wr,
            "ident": np.eye(128, dtype=np.float32),
        }
        if not fast:
            m["ones"] = np.ones((128, N_NT, 2), dtype=np.float32)
        if with_bq:
            m["bq"] = bq
        if with_br:
            m["br"] = br
        in_maps.append(m)

    res = run_bass_kernel_spmd(nc, in_maps, list(range(N_CORES)))
    y = np.concatenate([res.results[i]["out"] for i in range(N_CORES)], axis=0)
    return y.reshape(B, H, W, C)

